# revision 1
# baseline (speedup 1.0000x reference)
"""GAT layer kernel for Trainium2 (8 NeuronCores, SPMD, no collectives).

Math (reference):
    att = h @ h.T / sqrt(256)
    A = softmax(where(adj>0, att, -9e15), axis=1)
    A = (A + I) * 0.5; rows < k (k = nnz(adj[:,0])) overwritten with I
    out = relu(A @ (h @ W.T + b))

Algorithm here (flash-style, attention matrix never materialized/scaled):
  - rows [0,k): out = relu(h@W.T + b)  (identity rows)
  - rows [k,N): out = relu(0.5*num/S + 0.5*h@W.T|row + b), where
        num = sum_j mask[i,j]*exp(att[i,j]) * (h@W.T)[j],
        S   = sum_j mask[i,j]*exp(att[i,j])
    Masking by multiply after exp (exact zeros); no row-max subtraction
    needed: att in [-7, 22] for this input family, exp stays in f32 range.
  - Transposed layout: each core computes att_T[j, i] for its own output
    rows i (sharded on host), j contracted over all 8192 via PSUM
    accumulation; numerator and denominator come from one matmul chain
    against [h_new | 1].

Sharding: identity rows and attention rows each split evenly across the 8
cores; every core runs the same NEFF on different input slices.
"""

import math
import os
import sys

for _p in ("/opt/trn_rl_repo", "/root/.axon_site/_ro/trn_rl_repo"):
    if os.path.isdir(_p) and _p not in sys.path:
        sys.path.append(_p)

import numpy as np
import orjson

import concourse.bass as bass
import concourse.tile as tile
from concourse import mybir

F32 = mybir.dt.float32
F16 = mybir.dt.float16
BF16 = mybir.dt.bfloat16
I8 = mybir.dt.int8

N = 8192
D = 256
NCORES = 8
NJC = N // 128  # 64 j-chunks
SCALE = 1.0 / 16.0


def _spill_waits(nc, max_sync=2):
    """Walrus rejects instructions with more sync commands than the lowered
    ISA struct can hold (2 for compute/DMA, 1 for NoOp/Drain). Tile can emit
    more. Move excess waits onto injected NoOps preceding the instruction
    (same engine, executes in order, so semantics are preserved)."""
    bir = orjson.loads(nc.to_json_bytes())
    for fn in bir["functions"]:
        for bb in fn["blocks"]:
            insts = bb.get("instructions") or []
            out = []
            for inst in insts:
                si = inst.get("sync_info")
                if si:
                    waits = si.get("on_wait") or []
                    upds = si.get("on_update") or []
                    lim = 1 if inst["opcode"] in ("NoOp", "Drain") else max_sync
                    cap = max(0, lim - len(upds))
                    if len(waits) > cap:
                        extra = waits[cap:]
                        si["on_wait"] = waits[:cap]
                        for ci, w in enumerate(extra):
                            out.append(
                                {
                                    "engine": inst["engine"],
                                    "ins": [],
                                    "outs": [],
                                    "name": f"{inst['name']}-sw{ci}",
                                    "opcode": "NoOp",
                                    "sync_info": {"on_wait": [w], "on_update": []},
                                    "debug": inst.get("debug", 0),
                                }
                            )
                out.append(inst)
            bb["instructions"] = out
    blob = orjson.dumps(bir)
    nc.to_json_bytes = lambda: blob


def _build(nid, nis, repeat=1, hnew_mode="compute", abl=(), depth=2, att_bufs=2, merge_ps=False, work_bufs=4):
    """Build the SPMD program. nid/nis = number of 128-row identity /
    attention sub-tiles per core. OWN = (nid+nis)*128 own rows per core.
    repeat: unroll the whole kernel body N times (benchmark use)."""
    nown = nid + nis
    own = nown * 128
    rpad = nis * 128

    nc = bass.Bass("TRN2", target_bir_lowering=False, debug=False, num_devices=NCORES)

    hT_d = nc.dram_tensor("hT", [D, N], F16, kind="ExternalInput").ap()
    hTo_d = nc.dram_tensor("hTo", [D, own], F16, kind="ExternalInput").ap()
    WT_d = nc.dram_tensor("WT", [D, 256], F16, kind="ExternalInput").ap()
    bb_d = nc.dram_tensor("bb", [128, 256], F32, kind="ExternalInput").ap()
    if nis:
        mT_d = nc.dram_tensor("mT", [N, rpad], I8, kind="ExternalInput").ap()
    if hnew_mode == "dram":
        hn_d = nc.dram_tensor("hn", [N, 257], BF16, kind="ExternalInput").ap()
    out_d = nc.dram_tensor("out", [own, 256], F32, kind="ExternalOutput").ap()

    with tile.TileContext(nc) as tc:
        pp = None  # set below
        with (
            tc.tile_pool(name="big", bufs=1) as big,
            tc.tile_pool(name="hnp", bufs=1) as hnp,
            tc.tile_pool(name="gout", bufs=1) as gout,
            tc.tile_pool(name="work", bufs=work_bufs) as work,
            tc.tile_pool(name="fin", bufs=2) as fin,
            tc.tile_pool(name="ps", bufs=2, space="PSUM") as pp0,
            tc.tile_pool(name="att_ps", bufs=att_bufs, space="PSUM") as app,
            tc.tile_pool(name="acc", bufs=1, space="PSUM") as accp,
        ):
            pp = app if merge_ps else pp0
            for _rep in range(repeat):
              # --- persistent loads ---
              # hT as 2 d-chunks x 4 column-chunks of 2048 (fewer DMAs --
              # HWDGE per-DMA overhead is ~0.5us)
              hTt = [[None] * 4 for _ in range(2)]
              for dchunk in range(2):
                  for cc in range(4):
                      t = big.tile([128, 2048], F16, tag=f"hT{dchunk}_{cc}")
                      nc.sync.dma_start(
                          t[:],
                          hT_d[
                              dchunk * 128 : (dchunk + 1) * 128,
                              cc * 2048 : (cc + 1) * 2048,
                          ],
                      )
                      hTt[dchunk][cc] = t
              hTo_t = []
              WT_t = []
              for dchunk in range(2):
                  t = big.tile([128, own], F16, tag=f"hTo{dchunk}")
                  nc.sync.dma_start(t[:], hTo_d[dchunk * 128 : (dchunk + 1) * 128, :])
                  hTo_t.append(t)
                  t = big.tile([128, 256], F16, tag=f"WT{dchunk}")
                  nc.sync.dma_start(t[:], WT_d[dchunk * 128 : (dchunk + 1) * 128, :])
                  WT_t.append(t)
              bb_t = big.tile([128, 256], F32, tag="bb")
              nc.sync.dma_start(bb_t[:], bb_d[:, :])

              def hT_slice(dchunk, jc):
                  return hTt[dchunk][jc // 16][:, (jc % 16) * 128 : (jc % 16 + 1) * 128]

              # --- own phase: h_new for own rows ---
              # identity tiles -> out rows directly; attention tiles -> g
              g_t = []
              if "no_own" in abl:
                  for t_i in range(nid, nown):
                      g = gout.tile([128, 256], F32, tag=f"g{t_i - nid}")
                      nc.vector.memset(g[:], 0.5)
                      g_t.append(g)
              for t_i in range(0 if "no_own" in abl else nown):
                  ps = pp.tile([128, 256], F32, tag="att_ps" if merge_ps else "hn_ps")
                  for dchunk in range(2):
                      nc.tensor.matmul(
                          ps[:],
                          hTo_t[dchunk][:, t_i * 128 : (t_i + 1) * 128],
                          WT_t[dchunk][:],
                          start=(dchunk == 0),
                          stop=(dchunk == 1),
                      )
                  if t_i < nid:
                      tmp = fin.tile([128, 256], F32, tag="idtmp")
                      nc.vector.tensor_tensor(
                          tmp[:], ps[:], bb_t[:], op=mybir.AluOpType.add
                      )
                      o_t = fin.tile([128, 256], F32, tag="ido")
                      nc.vector.tensor_scalar_max(o_t[:], tmp[:], 0.0)
                      nc.sync.dma_start(
                          out_d[t_i * 128 : (t_i + 1) * 128, :], o_t[:]
                      )
                  else:
                      g = gout.tile([128, 256], F32, tag=f"g{t_i - nid}")
                      nc.vector.scalar_tensor_tensor(
                          g[:],
                          ps[:],
                          0.5,
                          bb_t[:],
                          op0=mybir.AluOpType.mult,
                          op1=mybir.AluOpType.add,
                      )
                      g_t.append(g)

              if nis:
                  # --- h_new phase: h_new_plus[jc] = [h@W.T | 1] bf16 ---
                  hnew = []
                  if hnew_mode == "dram":
                      hnb = hnp.tile([128, NJC * 257], BF16, tag="hnewbig")
                      hn_r = hn_d.rearrange("(a p) w -> p a w", p=128)
                      for c2 in range(2):
                          nc.sync.dma_start(
                              hnb[:, c2 * 32 * 257 : (c2 + 1) * 32 * 257].rearrange(
                                  "p (a w) -> p a w", a=32
                              ),
                              hn_r[:, c2 * 32 : (c2 + 1) * 32, :],
                          )
                      hnew = [hnb[:, jc * 257 : (jc + 1) * 257] for jc in range(NJC)]
                  for jc in range(NJC if hnew_mode != "dram" else 0):
                      hp = hnp.tile([128, 257], BF16, tag=f"hnew{jc}")
                      if False:
                          pass
                      else:
                          ps = pp.tile([128, 256], F32, tag="att_ps" if merge_ps else "hn_ps")
                          for dchunk in range(2):
                              nc.tensor.matmul(
                                  ps[:],
                                  hT_slice(dchunk, jc),
                                  WT_t[dchunk][:],
                                  start=(dchunk == 0),
                                  stop=(dchunk == 1),
                              )
                          if jc % 2 == 0:
                              nc.vector.tensor_copy(hp[:, 0:256], ps[:])
                          else:
                              nc.scalar.copy(hp[:, 0:256], ps[:])
                          nc.vector.memset(hp[:, 256:257], 1.0)
                      hnew.append(hp)

                  # --- mask preload: [128, 64*rpad] i8, 4 big DMAs ---
                  if "no_att" in abl:
                      pass
                  elif "no_mask_dma" not in abl:
                      mbig = big.tile([128, NJC * rpad], I8, tag="mbig")
                      mT_r = mT_d.rearrange("(a p) w -> p a w", p=128)
                      for c4 in range(4):
                          nc.sync.dma_start(
                              mbig[:, c4 * 16 * rpad : (c4 + 1) * 16 * rpad].rearrange(
                                  "p (a w) -> p a w", a=16
                              ),
                              mT_r[:, c4 * 16 : (c4 + 1) * 16, :],
                          )

                  # --- attention phase ---
                  for ig in range(0 if "no_att" in abl else math.ceil(nis / 4)):
                      s0 = ig * 4
                      s1 = min(s0 + 4, nis)
                      iw = (s1 - s0) * 128  # width of this i-group
                      i_lo = s0 * 128
                      s_active = [s0] if "one_second" in abl else list(range(s0, s1))
                      acc = {}
                      for s in s_active:
                          acc_t = accp.tile([128, 257], F32, tag=f"acc{s - s0}")
                          acc[s - s0] = acc_t
                      # software pipeline: 2nd matmul for jc emitted DEPTH
                      # iterations later so PE doesn't wait on exp->mask chain
                      DEPTH = depth
                      pend = []

                      def emit_second(jc, em_t):
                          for s in s_active:
                              nc.tensor.matmul(
                                  acc[s - s0][:],
                                  em_t[:, (s - s0) * 128 : (s - s0 + 1) * 128],
                                  hnew[jc][:],
                                  start=(jc == 0),
                                  stop=(jc == NJC - 1),
                              )

                      for jc in range(NJC):
                          aps = app.tile([128, 512], F32, tag="att_ps")
                          ndch = 1 if "one_dchunk" in abl else 2
                          for dchunk in range(ndch):
                              nc.tensor.matmul(
                                  aps[:, 0:iw],
                                  hT_slice(dchunk, jc),
                                  hTo_t[dchunk][
                                      :, (nid * 128 + i_lo) : (nid * 128 + i_lo + iw)
                                  ],
                                  start=(dchunk == 0),
                                  stop=(dchunk == ndch - 1),
                              )
                          e_t = work.tile([128, 512], BF16, tag="e")
                          nc.scalar.activation(
                              e_t[:, 0:iw],
                              aps[:, 0:iw],
                              mybir.ActivationFunctionType.Copy
                              if "no_exp" in abl
                              else mybir.ActivationFunctionType.Exp,
                              scale=SCALE,
                          )
                          if "no_mask_dma" in abl:
                              if jc == 0:
                                  mfix = big.tile([128, 512], I8, tag="mfix")
                                  nc.vector.memset(mfix[:, 0:iw], 1)
                              m_sl = mfix[:, 0:iw]
                          else:
                              m_sl = mbig[:, jc * rpad + i_lo : jc * rpad + i_lo + iw]
                          if "no_mask_tt" in abl:
                              em_t = e_t
                          else:
                              em_t = work.tile([128, 512], BF16, tag="em")
                              nc.vector.tensor_tensor(
                                  em_t[:, 0:iw], e_t[:, 0:iw], m_sl,
                                  op=mybir.AluOpType.mult,
                              )
                          pend.append((jc, em_t))
                          if len(pend) > DEPTH:
                              emit_second(*pend.pop(0))
                      for item in pend:
                          emit_second(*item)
                      for s in s_active:
                          a = acc[s - s0]
                          recip = fin.tile([128, 1], F32, tag="recip")
                          nc.vector.reciprocal(recip[:], a[:, 256:257])
                          hr = fin.tile([128, 1], F32, tag="hr")
                          nc.vector.tensor_scalar_mul(hr[:], recip[:], 0.5)
                          tmp = fin.tile([128, 256], F32, tag="atmp")
                          nc.vector.scalar_tensor_tensor(
                              tmp[:],
                              a[:, 0:256],
                              hr[:],
                              g_t[s][:],
                              op0=mybir.AluOpType.mult,
                              op1=mybir.AluOpType.add,
                          )
                          o_t = fin.tile([128, 256], F32, tag="ao")
                          nc.vector.tensor_scalar_max(o_t[:], tmp[:], 0.0)
                          nc.sync.dma_start(
                              out_d[(nid + s) * 128 : (nid + s + 1) * 128, :], o_t[:]
                          )

    _spill_waits(nc)
    return nc


_CACHE = {}


def _prepare(h, adj, W, b):
    """Host-side sharding. Returns (nc, in_maps, assemble) where assemble
    takes the list of per-core 'out' arrays and produces the full output."""
    h = np.asarray(h, dtype=np.float32)
    adj = np.asarray(adj)
    W = np.asarray(W, dtype=np.float32)
    b = np.asarray(b, dtype=np.float32)

    k = int(np.count_nonzero(adj[:, 0]))
    nid = (k + NCORES * 128 - 1) // (NCORES * 128)  # id 128-tiles per core
    nis = (N - k + NCORES * 128 - 1) // (NCORES * 128)  # att 128-tiles per core
    key = (nid, nis)
    if key not in _CACHE:
        _CACHE[key] = _build(nid, nis)
    nc = _CACHE[key]

    kid = nid * 128  # padded id rows per core
    rpad = nis * 128  # padded att rows per core
    own = kid + rpad

    hT16 = np.ascontiguousarray(h.T).astype(np.float16)  # [D, N]
    WT16 = np.ascontiguousarray(W.T).astype(np.float16)
    bb = np.broadcast_to(b, (128, 256)).astype(np.float32).copy()
    adj8 = (adj != 0).view(np.int8) if adj.dtype == np.bool_ else (adj != 0)
    adj8 = adj8.view(np.int8) if adj8.dtype == np.bool_ else adj8.astype(np.int8)

    in_maps = []
    row_lists = []
    for c in range(NCORES):
        id_rows = np.arange(c * kid, (c + 1) * kid)
        id_valid = id_rows < k
        id_rows = np.where(id_valid, id_rows, 0)
        att_rows = np.arange(k + c * rpad, k + (c + 1) * rpad)
        att_valid = att_rows < N
        att_rows_c = np.where(att_valid, att_rows, 0)
        rows = np.concatenate([id_rows, att_rows_c])
        row_lists.append((id_rows, id_valid, att_rows_c, att_valid))

        hTo = np.ascontiguousarray(hT16[:, rows])  # [D, own] fp16
        im = {"hT": hT16, "hTo": hTo, "WT": WT16, "bb": bb}
        if nis:
            mT = np.zeros((N, rpad), dtype=np.int8)
            nval = int(att_valid.sum())
            if nval:
                mT[:, :nval] = adj8[att_rows_c[:nval], :].T
            im["mT"] = mT
        in_maps.append(im)

    def assemble(outs):
        out = np.empty((N, 256), dtype=np.float32)
        for c in range(NCORES):
            id_rows, id_valid, att_rows_c, att_valid = row_lists[c]
            o = outs[c]
            if id_valid.any():
                out[id_rows[id_valid]] = o[:kid][id_valid]
            if att_valid.any():
                out[att_rows_c[att_valid]] = o[kid:][att_valid]
        return out

    return nc, in_maps, assemble


def kernel(h, adj, W, b):
    nc, in_maps, assemble = _prepare(h, adj, W, b)

    from concourse.bass_utils import run_bass_kernel_spmd

    res = run_bass_kernel_spmd(nc, in_maps, core_ids=list(range(NCORES)))
    return assemble([res.results[c]["out"] for c in range(NCORES)])

